# revision 30
# baseline (speedup 1.0000x reference)
"""Causal attention (B=4, T=2048, D=1024, fp32) on 8 TRN2 NeuronCores.

Sharding: core c -> batch b = c//2, q-row parity h = c%2 (rows x[b, h::2]).
The interleaved row split makes every core's causal block structure identical,
so one SPMD program serves all 8 cores; causality inside the diagonal 256-wide
k-block is enforced with a per-core additive mask input.

Key algebraic trick: scores = (x_q Wq^T)(x Wk^T)^T = x_q (Wq^T Wk) x^T, so the
host passes M = Wq^T @ Wk as the projection matrix and the device contracts
qm = x_q M directly against the resident x^T — the whole K projection phase
(and one full x stream) vanishes.

Per core: qm projection into a resident SBUF tile (no DRAM bounce); then an
interleaved V+attention stream over ascending 128-row q-tiles — v rows
2t,2t+1 are projected just before attention tile t, so every tile's softmax
chain (DVE/ACT latency) hides inside the dense V matmuls. Per tile: scores
over the causal k-prefix, softmax (negated row-max bias into in-place Exp
with accumulated row sum), PE-transpose of the weights, AV accumulation,
1/sum normalization. All matmuls run in float32r (full-rate fp32 streaming
mode, ~2e-4 relative error).

All loads are plain resident-tile DMAs on the SP HWDGE queue in prefetch
order — no slot recycling, so nothing ever blocks the load queue. The only
writebacks (outputs) go through GPSIMD/SWDGE.
"""
import os
import numpy as np

B, T, D = 4, 2048, 1024
TQ = T // 2          # local q rows per core
P = 128              # partitions
DC = D // P          # d-chunks (contraction)
KB = 256             # k-block width for scores (causal prefix granularity)
XC = 256             # x_q chunk width (DMA + Q-phase i-tiles)
NQT = TQ // P        # q-tiles per core
SCALE = 1.0 / 32.0   # 1/sqrt(D)
NEG = -1.0e30

_CACHE = {}


def _build_nc():
    import concourse.tile as tile
    import concourse.mybir as mybir
    from concourse import bacc
    from concourse.masks import make_identity
    from contextlib import ExitStack

    F32 = mybir.dt.float32
    F32R = mybir.dt.float32r
    Exp = mybir.ActivationFunctionType.Exp
    X = mybir.AxisListType.X

    nc = bacc.Bacc("TRN2", target_bir_lowering=False, debug=False, num_devices=8,
                   dynamic_dma_scratch_size=4096)

    xq = nc.dram_tensor("xq", [D, TQ], F32R, kind="ExternalInput").ap()
    xkv = nc.dram_tensor("xkv", [D, T], F32R, kind="ExternalInput").ap()
    wq = nc.dram_tensor("wq", [D, D], F32R, kind="ExternalInput").ap()  # = Wq^T Wk
    wv = nc.dram_tensor("wv", [D, D], F32R, kind="ExternalInput").ap()
    maskd = nc.dram_tensor("mask", [P, KB], F32, kind="ExternalInput").ap()
    outd = nc.dram_tensor("out", [TQ, D], F32, kind="ExternalOutput").ap()

    def chunked(ap):
        return ap.rearrange("(c p) n -> p c n", p=P)

    with tile.TileContext(nc) as tc, ExitStack() as top:
        small = top.enter_context(tc.tile_pool(name="small", bufs=1, side="left"))
        kvx = top.enter_context(tc.tile_pool(name="kvx", bufs=1, side="left"))
        xkv_sb = kvx.tile([P, DC, T], F32R)           # x^T resident, 8MB
        qt_sb = kvx.tile([P, DC, TQ], F32R)           # qmT resident, 4MB
        ps = top.enter_context(tc.tile_pool(name="ps", bufs=2, space="PSUM"))
        # Right-side stack, opened in reverse order of release.
        wvp_es, xqp_es, wqp_es = (ExitStack() for _ in range(3))
        wvp = wvp_es.enter_context(tc.tile_pool(name="wvp", bufs=1, side="right"))
        xqp = xqp_es.enter_context(tc.tile_pool(name="xqp", bufs=1, side="right"))
        wqp = wqp_es.enter_context(tc.tile_pool(name="wqp", bufs=1, side="right"))

        # SP load queue in prefetch priority order; every DMA targets a
        # resident tile, so the queue never blocks on buffer recycling.
        wq_sb = wqp.tile([P, DC, D], F32R)
        xq_sb = xqp.tile([P, DC, TQ], F32R)
        wv_sb = wvp.tile([P, DC, D], F32R)
        # Interleave wq e-slices with xq chunks so Q-group availability
        # tracks the wavefront consumption order below.
        def wq_slice(ec):
            nc.sync.dma_start(wq_sb[:, :, ec * P:(ec + 1) * P],
                              chunked(wq[:, ec * P:(ec + 1) * P]))

        def xq_chunk(it):
            nc.sync.dma_start(xq_sb[:, :, it * XC:(it + 1) * XC],
                              chunked(xq[:, it * XC:(it + 1) * XC]))

        wq_slice(0); xq_chunk(0); wq_slice(1); wq_slice(2); xq_chunk(1)
        wq_slice(3); wq_slice(4); xq_chunk(2); wq_slice(5); wq_slice(6)
        xq_chunk(3); wq_slice(7)
        # modeled arrival times (1.45us per 0.5MB unit) for ordering below
        wq_at = [1.45, 5.8, 7.25, 11.6, 13.05, 17.4, 18.85, 23.2]
        xq_at = [4.35, 10.15, 15.95, 21.75]
        nc.sync.dma_start(wv_sb[:, :, 0:512], chunked(wv[:, 0:512]))
        nc.sync.dma_start(xkv_sb[:, :, 0:512], chunked(xkv[:, 0:512]))
        nc.sync.dma_start(wv_sb[:, :, 512:1024], chunked(wv[:, 512:1024]))
        for ck in range(1, 4):
            nc.sync.dma_start(xkv_sb[:, :, ck * 512:(ck + 1) * 512],
                              chunked(xkv[:, ck * 512:(ck + 1) * 512]))
        mask_sb = small.tile([P, KB], F32)
        nc.sync.dma_start(mask_sb[:], maskd[:])
        ident = small.tile([P, P], F32)
        make_identity(nc, ident[:])
        identr = small.tile([P, P], F32R)
        nc.scalar.copy(identr[:], ident[:])

        # ---- Phase Q: qmT[dd,i] = M^T @ x_q^T into resident qt_sb ----
        groups = sorted(((it, ec) for it in range(TQ // XC) for ec in range(DC)),
                        key=lambda g: (max(xq_at[g[0]], wq_at[g[1]]), g[0], g[1]))
        for it, ec in groups:
            if True:
                ps_q = ps.tile([P, XC], F32, tag="proj", name="ps_q")
                for dc in range(DC):
                    nc.tensor.matmul(
                        ps_q[:], wq_sb[:, dc, ec * P:(ec + 1) * P],
                        xq_sb[:, dc, it * XC:(it + 1) * XC],
                        start=(dc == 0), stop=(dc == DC - 1))
                nc.scalar.copy(qt_sb[:, ec, it * XC:(it + 1) * XC], ps_q[:])
        wqp_es.close()
        xqp_es.close()

        # ---- Phases V+A interleaved: produce v rows 2t,2t+1 then run
        # attention tile t (ascending), so each tile's softmax chain hides
        # inside the dense V matmul stream. ----
        vp = top.enter_context(tc.tile_pool(name="vp", bufs=1, side="left"))
        v_sb = vp.tile([P, T // P, D], F32R)          # v[j,e]  8MB
        with tc.tile_pool(name="pha", bufs=2, side="left") as pha:
            for ti in range(NQT):
                nkb = ti + 1           # causal prefix length in 256-wide k-blocks
                L = nkb * KB
                scores = pha.tile([P, T], F32R, tag="scores", bufs=1)
                for kb in range(nkb):
                    ps_s = ps.tile([P, KB], F32, tag="ps_s", bufs=3)
                    for dc in range(DC):
                        nc.tensor.matmul(
                            ps_s[:], qt_sb[:, dc, ti * P:(ti + 1) * P],
                            xkv_sb[:, dc, kb * KB:(kb + 1) * KB],
                            start=(dc == 0), stop=(dc == DC - 1))
                    if kb == nkb - 1:
                        nc.vector.tensor_add(
                            scores[:, kb * KB:(kb + 1) * KB], ps_s[:], mask_sb[:])
                    else:
                        nc.vector.tensor_copy(scores[:, kb * KB:(kb + 1) * KB], ps_s[:])
                # v rows 2t,2t+1 here: dense PE work that hides this tile's
                # softmax chain (DVE/ACT) before its transposes and AV.
                for jc in (2 * ti, 2 * ti + 1):
                    for et in range(D // 512):
                        ps_v = ps.tile([P, 512], F32, tag="proj", name="ps_v")
                        for dc in range(DC):
                            nc.tensor.matmul(
                                ps_v[:], xkv_sb[:, dc, jc * P:(jc + 1) * P],
                                wv_sb[:, dc, et * 512:(et + 1) * 512],
                                start=(dc == 0), stop=(dc == DC - 1))
                        nc.scalar.copy(v_sb[:, jc, et * 512:(et + 1) * 512], ps_v[:])
                nmax = pha.tile([P, 1], F32, tag="nmax")
                nc.vector.reduce_max(nmax[:], scores[:, :L], axis=X, negate=True)
                nmaxs = pha.tile([P, 1], F32, tag="nmaxs")
                nc.vector.tensor_scalar_mul(nmaxs[:], nmax[:], SCALE)
                rsum = pha.tile([P, 1], F32, tag="rsum")
                nc.scalar.activation(scores[:, :L], scores[:, :L], Exp,
                                     bias=nmaxs[:], scale=SCALE, accum_out=rsum[:])
                rinv = pha.tile([P, 1], F32, tag="rinv")
                nc.vector.reciprocal(rinv[:], rsum[:])
                wT_sb = pha.tile([P, T // P, P], F32R, tag="wT_sb", bufs=1)
                for jc in range(nkb * (KB // P)):
                    ps_t = ps.tile([P, P], F32R, tag="ps_t")
                    nc.tensor.transpose(ps_t[:], scores[:, jc * P:(jc + 1) * P],
                                        identr[:])
                    # alternate drain engines so the 2 ps_t slots recycle
                    # faster than the PE produces transposes
                    if jc % 2 == 0:
                        nc.scalar.copy(wT_sb[:, jc], ps_t[:])
                    else:
                        nc.vector.tensor_copy(wT_sb[:, jc], ps_t[:])
                njc = nkb * (KB // P)
                for et in range(D // 512):
                    ps_o = ps.tile([P, 512], F32, tag="ps_o", bufs=1)
                    for jc in range(njc):
                        nc.tensor.matmul(
                            ps_o[:], wT_sb[:, jc], v_sb[:, jc, et * 512:(et + 1) * 512],
                            start=(jc == 0), stop=(jc == njc - 1))
                    o_sb = pha.tile([P, 512], F32, tag="o_sb", bufs=2)
                    nc.vector.tensor_scalar_mul(o_sb[:], ps_o[:], rinv[:])
                    wb = nc.scalar if ti == NQT - 1 else nc.gpsimd
                    wb.dma_start(
                        outd[ti * P:(ti + 1) * P, et * 512:(et + 1) * 512], o_sb[:])
        wvp_es.close()

    nc.compile()
    return nc


def _get_nc():
    if "nc" not in _CACHE:
        _CACHE["nc"] = _build_nc()
    return _CACHE["nc"]


def _mask_for(h: int) -> np.ndarray:
    r = np.arange(P)[:, None]
    jj = np.arange(KB)[None, :]
    return np.where(jj <= 2 * r + h, 0.0, NEG).astype(np.float32)


def kernel(inputs, Wq, Wk, Wv):
    from concourse.bass_utils import run_bass_kernel_spmd

    x = np.ascontiguousarray(np.asarray(inputs, dtype=np.float32))
    wq_np = np.asarray(Wq, dtype=np.float32)
    wk_np = np.asarray(Wk, dtype=np.float32)
    M = np.ascontiguousarray(wq_np.T @ wk_np)                 # [d, dd]
    wvT = np.ascontiguousarray(np.asarray(Wv, dtype=np.float32).T)

    nc = _get_nc()
    in_maps = []
    for c in range(8):
        b, h = c // 2, c % 2
        xb = x[b]
        in_maps.append({
            "xq": np.ascontiguousarray(xb[h::2].T),
            "xkv": np.ascontiguousarray(xb.T),
            "wq": M, "wv": wvT,
            "mask": _mask_for(h),
        })
    trace = bool(int(os.environ.get("KBENCH_TRACE", "0")))
    res = run_bass_kernel_spmd(nc, in_maps, list(range(8)), trace=trace)
    _CACHE["last_results"] = res

    out = np.empty((B, T, D), dtype=np.float32)
    for c in range(8):
        b, h = c // 2, c % 2
        out[b, h::2] = res.results[c]["out"]
    return out
